# revision 17
# baseline (speedup 1.0000x reference)
"""LightGCN-style GNN message passing on 8 Trainium2 NeuronCores (v3).

Algorithm (matches the reference):
    deg  = bincount(dst);  dinv = rsqrt(max(deg, 1))
    x_{l+1} = dinv * (A @ (dinv * x_l))          (3 layers, A = binary adjacency)
    z_l = l2_normalize(x_l);  Z = concat(z_0..z_3);  Y = Z @ W.T + b
    return Y[senders], Y[receivers]

Factorization: with xs_l = dinv * x_l, messages need no per-edge scale and
l2_normalize(xs_l) == l2_normalize(x_l); only xs tables are materialized (bf16).
xs_0 = dinv*emb is part of the host-side input prep (alongside deg/dinv and
the index schedules); the device does all per-edge gather/scatter, the three
propagation layers, the normalizations and the output MLP.

Sharding: destination-sharded.  Core i owns N/8 dst rows, split into NP=4
pieces [31,31,31,5] (blocks of 128).  Each xs table's AllGather is issued per
piece as soon as that piece's rows are computed, so collectives overlap the
remaining blocks' compute; the tiny last piece minimizes the tail the next
layer's first phase must wait for.  Tables 1,2 are AllGathered (layers 2,3
read them); table 3 is not - the final stage gathers all four z rows from the
LOCAL shard of the node that owns each output row (all tables share the same
dst sharding), and the host reassembles outputs by ownership.

Edge schedule: per core, 4 gather streams (one per src piece).  Within a
stream, edges are grouped by dst block with cells padded to 16-index
granularity (L_pb = max-over-cores, >=128).  Fixed 128-slot matmul windows
run over each stream; a window that straddles a cell boundary issues two
one-hot matmuls (S built on DVE via iota + is_equal; -1 matches nothing),
accumulating the segment-sum in PSUM per dst block on PE.  Gather
descriptors (~7.7ns each, descriptor-count-bound) are the kernel's floor.
"""

import numpy as np
import ml_dtypes

import concourse.bacc as bacc
import concourse.mybir as mybir
import concourse.tile as tile

F32 = mybir.dt.float32
BF16 = mybir.dt.bfloat16
I16 = mybir.dt.int16
I32 = mybir.dt.int32

D = 128             # feature dim
NL = 3              # message passing layers
NC = 8              # cores
BLK = 128           # dst block (psum partition dim)
NP = 4              # src/dst pieces (gather windows + AllGather pipeline)


def _ceil(a, b):
    return (a + b - 1) // b


class Cfg:
    def __init__(self, N, E, NOUT, GCALL=2048):
        self.N = N
        self.E = E
        self.NOUT = NOUT
        self.GCALL = GCALL
        self.PER = N // NC
        self.NB = _ceil(self.PER, BLK)
        self.SEG = self.NB * BLK
        # pieces: equal-ish but last one small (cheap AllGather tail);
        # per-piece window rows (NC*PB*BLK) must fit int16
        big = min(self.NB - 1, 32767 // (NC * BLK))
        nbig = NP - 1
        while (self.NB - nbig * big) < 1 or (self.NB - nbig * big) > big:
            big -= 1
        self.PB = [big] * nbig + [self.NB - nbig * big]
        self.PSTART = np.concatenate([[0], np.cumsum(self.PB)[:-1]]).astype(int)
        self.PR = [pb * BLK for pb in self.PB]          # rows/piece/core
        self.WROWS = [NC * pr for pr in self.PR]        # gather window rows
        assert max(self.WROWS) <= 32767, "int16 gather index overflow"
        assert self.SEG <= 32767
        self.OPC = NOUT // NC


FULL = Cfg(N=100000, E=1600000, NOUT=16384)


def _wrap16(idx):
    """int16 [L] -> [128, L//16] wrapped in 16 partitions, replicated x8."""
    return np.tile(idx.reshape(-1, 16).T, (8, 1)).copy()


def _prep(cfg, emb, edge_index, senders, receivers, W, b):
    N, E, PER, SEG, NB = cfg.N, cfg.E, cfg.PER, cfg.SEG, cfg.NB
    PB, PSTART, PR = cfg.PB, np.asarray(cfg.PSTART), cfg.PR
    src = np.asarray(edge_index[0], np.int64)
    dst = np.asarray(edge_index[1], np.int64)
    senders = np.asarray(senders, np.int64)
    receivers = np.asarray(receivers, np.int64)
    bias = np.asarray(b, np.float32)    # `b` is shadowed by loop vars below

    deg = np.bincount(dst, minlength=N).astype(np.float32)
    deg = np.maximum(deg, 1.0)
    dinv = (1.0 / np.sqrt(deg)).astype(np.float32)
    dinv2 = (dinv * dinv).astype(np.float32)

    piece_of_block = np.zeros(NB, np.int64)
    for q in range(NP):
        piece_of_block[PSTART[q]:PSTART[q] + PB[q]] = q
    PRa = np.asarray(PR)

    def node_piece_idx(x):
        """node id -> (piece, in-window row)"""
        ci = x // PER
        r = x % PER
        blk = r // BLK
        p = piece_of_block[blk]
        sidx = ci * PRa[p] + (blk - PSTART[p]) * BLK + (r % BLK)
        return p, sidx

    p_s, sidx = node_piece_idx(src)
    ci_d = dst // PER
    r_d = dst % PER
    b_d = r_d // BLK
    dloc = r_d % BLK

    # --- cell sizes & stream layout (shared across cores) ------------------
    key = (ci_d * NP + p_s) * NB + b_d
    counts = np.bincount(key, minlength=NC * NP * NB).reshape(NC, NP, NB)
    L_pb = np.maximum(_ceil(counts.max(axis=0), 16) * 16, BLK)     # [NP, NB]
    O_pb = np.zeros((NP, NB), np.int64)                            # cell base
    stream_len = np.zeros(NP, np.int64)
    for p in range(NP):
        O_pb[p] = np.concatenate([[0], np.cumsum(L_pb[p])[:-1]])
        stream_len[p] = _ceil(int(L_pb[p].sum()), BLK) * BLK
    W_p = (stream_len // BLK).astype(int)
    W_off = np.concatenate([[0], np.cumsum(W_p)[:-1]]).astype(int)
    Wtot = int(W_p.sum())
    stream_base = np.concatenate([[0], np.cumsum(stream_len)[:-1]]).astype(int)
    TOTLEN = int(stream_len.sum())
    off16 = (stream_base // 16).astype(int)

    # window -> start block / straddle (shared)
    bstart_w = np.zeros(Wtot, np.int64)
    straddle_w = np.zeros(Wtot, bool)
    for p in range(NP):
        ends = np.cumsum(L_pb[p])                  # cell end positions
        L_real = int(L_pb[p].sum())
        for wl in range(W_p[p]):
            s0 = wl * BLK
            if s0 >= L_real:
                bstart_w[W_off[p] + wl] = NB       # pure-pad tail window
                continue
            b0 = int(np.searchsorted(ends, s0, side="right"))
            send = min(s0 + BLK - 1, L_real - 1)
            b1 = int(np.searchsorted(ends, send, side="right"))
            assert b1 - b0 <= 1, "window spans >2 cells"
            bstart_w[W_off[p] + wl] = b0
            straddle_w[W_off[p] + wl] = b1 != b0

    # schedule: per phase (dst block) the matmul ops in program order
    # op = [p, w_global, which, start, stop]; target psum = b + which
    sched = [[] for _ in range(NB)]
    for bq in range(NB):
        for p in range(NP):
            wg = W_off[p] + np.nonzero(
                bstart_w[W_off[p]:W_off[p] + W_p[p]] == bq)[0]
            for w in wg:
                sched[bq].append([p, int(w), 0, False, False])
                if straddle_w[w]:
                    sched[bq].append([p, int(w), 1, False, False])
    first = {}
    last = {}
    for bq in range(NB):
        for oi, op in enumerate(sched[bq]):
            tgt = bq + op[2]
            if tgt not in first:
                first[tgt] = (bq, oi)
            last[tgt] = (bq, oi)
    for tgt, (bq, oi) in first.items():
        sched[bq][oi][3] = True
    for tgt, (bq, oi) in last.items():
        sched[bq][oi][4] = True
    assert set(first) == set(range(NB))

    # --- per-core edge index / edloc arrays --------------------------------
    order = np.argsort(key, kind="stable")
    cnt_flat = counts.reshape(-1)
    starts_flat = np.concatenate([[0], np.cumsum(cnt_flat)[:-1]])
    rank = np.arange(E, dtype=np.int64) - starts_flat[key[order]]
    p_o = p_s[order]
    b_o = b_d[order]
    pos = O_pb[p_o, b_o] + rank                    # in-stream position
    core_o = ci_d[order]
    sidx_o = sidx[order]
    dloc_o = dloc[order]

    eidx_arrs, edloc_arrs = [], []
    for i in range(NC):
        m = core_o == i
        ia = np.zeros(TOTLEN, np.int16)
        ia[stream_base[p_o[m]] + pos[m]] = sidx_o[m].astype(np.int16)
        ed = np.full((BLK, 2 * Wtot), -1.0, np.float32)
        wg = W_off[p_o[m]] + pos[m] // BLK
        j = pos[m] % BLK
        which = (b_o[m] != bstart_w[wg]).astype(np.int64)
        ed[j, 2 * wg + which] = dloc_o[m]
        eidx_arrs.append(_wrap16(ia))
        edloc_arrs.append(ed)

    # --- output-row schedule (ownership: node's dst shard) -----------------
    NOUT = cfg.NOUT
    ids_all = np.concatenate([senders, receivers])          # [2*NOUT]
    owner = ids_all // PER
    r_all = ids_all % PER                                   # local padded row
    cnts = np.bincount(owner, minlength=NC)
    OUT_T = _ceil(int(cnts.max()), BLK)
    OUTLEN = OUT_T * BLK
    fidx_arrs, pos_arrs = [], []
    for i in range(NC):
        sel = np.nonzero(owner == i)[0]
        ia = np.zeros(OUTLEN, np.int16)
        ia[:len(sel)] = r_all[sel].astype(np.int16)
        fidx_arrs.append(_wrap16(ia))
        pos_arrs.append(sel)

    # --- per-core dense inputs --------------------------------------------
    xs0 = (emb * dinv[:, None]).astype(ml_dtypes.bfloat16)  # host input prep
    xs0_pad = np.zeros((NC, SEG, D), ml_dtypes.bfloat16)
    for i in range(NC):
        xs0_pad[i, :PER] = xs0[PER * i:PER * (i + 1)]
    # piece-major / core-major full table (matches AllGather output layout)
    xs0_full = np.concatenate(
        [xs0_pad[:, PSTART[q] * BLK:(PSTART[q] + PB[q]) * BLK, :]
         .reshape(-1, D) for q in range(NP)], axis=0)

    in_maps = []
    for i in range(NC):
        dv2 = np.zeros(SEG, np.float32)
        dv2[:PER] = dinv2[PER * i:PER * (i + 1)]
        in_maps.append({
            "xs0_full": xs0_full,
            "xs0_own": np.ascontiguousarray(xs0_pad[i]),
            "dinv2_col": dv2.reshape(NB, BLK).T.copy(),
            "eidx": eidx_arrs[i],
            "edloc": edloc_arrs[i],
            "fidx": fidx_arrs[i],
            "wt": np.ascontiguousarray(W.T).astype(ml_dtypes.bfloat16),
            "bb": np.broadcast_to(bias, (BLK, 4 * D)).astype(np.float32).copy(),
        })

    meta = {
        "sched": sched, "W_p": W_p, "W_off": W_off, "Wtot": Wtot,
        "stream_len": stream_len.astype(int), "off16": off16,
        "TOTLEN": TOTLEN, "OUT_T": OUT_T, "OUTLEN": OUTLEN,
    }
    return in_maps, meta, pos_arrs


def unshard(cfg, yvs, pos_arrs):
    """Reassemble (senders, receivers) outputs from per-core y tensors."""
    flat = np.empty((2 * cfg.NOUT, 4 * D), np.float32)
    for i in range(NC):
        sel = pos_arrs[i]
        flat[sel] = yvs[i][:len(sel)]
    return flat[:cfg.NOUT], flat[cfg.NOUT:]


def _build(cfg, meta, single=False, repeat=1, gbufs=4):
    SEG, NB, GCALL = cfg.SEG, cfg.NB, cfg.GCALL
    PB, PSTART, WROWS = cfg.PB, cfg.PSTART, cfg.WROWS
    sched = meta["sched"]
    W_p = meta["W_p"]
    W_off = meta["W_off"]
    Wtot = meta["Wtot"]
    stream_len = meta["stream_len"]
    off16 = meta["off16"]
    TOTLEN = meta["TOTLEN"]
    OUT_T = meta["OUT_T"]
    OUTLEN = meta["OUTLEN"]
    CPB = GCALL // BLK                              # windows per gather call
    NTOT = sum(WROWS)
    WOFFR = np.concatenate([[0], np.cumsum(WROWS)[:-1]]).astype(int)
    MAXSB = _ceil(max(PB), 2)                       # sub-slab blocks (SBUF)

    nc = bacc.Bacc("TRN2", target_bir_lowering=False, debug=False,
                   enable_asserts=False, num_devices=(1 if single else NC))

    xs0_full = nc.dram_tensor("xs0_full", [NTOT, D], BF16, kind="ExternalInput")
    xs0_own = nc.dram_tensor("xs0_own", [SEG, D], BF16, kind="ExternalInput")
    dinv2_col = nc.dram_tensor("dinv2_col", [128, NB], F32, kind="ExternalInput")
    eidx = nc.dram_tensor("eidx", [128, TOTLEN // 16], I16, kind="ExternalInput")
    edloc = nc.dram_tensor("edloc", [128, 2 * Wtot], F32, kind="ExternalInput")
    fidx = nc.dram_tensor("fidx", [128, OUTLEN // 16], I16, kind="ExternalInput")
    wt = nc.dram_tensor("wt", [4 * D, 4 * D], BF16, kind="ExternalInput")
    bb = nc.dram_tensor("bb", [128, 4 * D], F32, kind="ExternalInput")
    y = nc.dram_tensor("y", [OUTLEN, 4 * D], F32, kind="ExternalOutput")

    xs_own = [None] + [nc.dram_tensor(f"xs_own{l}", [SEG, D], BF16)
                       for l in range(1, NL + 1)]
    xs_piece = [[nc.dram_tensor(f"xs_p{l}_{q}", [WROWS[q], D], BF16,
                                addr_space="Shared") for q in range(NP)]
                for l in range(1, NL)]              # tables 1..NL-1 only
    RG = [list(range(NC))]

    def allgather(l, q):
        rows = slice(int(PSTART[q]) * BLK, (int(PSTART[q]) + PB[q]) * BLK)
        if single:
            nc.sync.dma_start(xs_piece[l - 1][q][:PB[q] * BLK, :],
                              xs_own[l][rows, :])
        else:
            nc.gpsimd.collective_compute(
                "AllGather", mybir.AluOpType.bypass, replica_groups=RG,
                ins=[xs_own[l][rows, :]], outs=[xs_piece[l - 1][q][:]])

    with tile.TileContext(nc) as tc:
        with tc.tile_pool(name="const", bufs=1) as cpool:
            eidx_sb = cpool.tile([128, TOTLEN // 16], I16, tag="eidx")
            for p in range(NP):       # piece 0 first so gathers start early
                lo = off16[p]
                hi = off16[p + 1] if p + 1 < NP else TOTLEN // 16
                nc.sync.dma_start(eidx_sb[:, lo:hi], eidx[:, lo:hi])
            edloc_sb = cpool.tile([128, 2 * Wtot], F32, tag="edloc")
            nc.sync.dma_start(edloc_sb[:], edloc[:])
            dinv2_sb = cpool.tile([128, NB], F32, tag="dinv2")
            nc.sync.dma_start(dinv2_sb[:], dinv2_col[:])
            fidx_sb = cpool.tile([128, OUTLEN // 16], I16, tag="fidx")
            nc.sync.dma_start(fidx_sb[:], fidx[:])
            wt_sb = cpool.tile([128, 4, 4 * D], BF16, tag="wt")
            nc.sync.dma_start(wt_sb[:], wt[:].rearrange("(l k) o -> k l o", k=128))
            bb_sb = cpool.tile([128, 4 * D], F32, tag="bb")
            nc.sync.dma_start(bb_sb[:], bb[:])

            iota_i = cpool.tile([128, 128], I32, tag="iota_i")
            nc.gpsimd.iota(iota_i[:], pattern=[[1, 128]], base=0,
                           channel_multiplier=0)
            iota_bf = cpool.tile([128, 128], BF16, tag="iota_bf")
            nc.vector.tensor_copy(iota_bf[:], iota_i[:])
            pidx_i = cpool.tile([128, 1], I32, tag="pidx_i")
            nc.gpsimd.iota(pidx_i[:], pattern=[[0, 1]], base=0,
                           channel_multiplier=1)
            pidx_f = cpool.tile([128, 1], F32, tag="pidx_f")
            nc.vector.tensor_copy(pidx_f[:], pidx_i[:])
            ident = cpool.tile([128, 128], BF16, tag="ident")
            nc.vector.tensor_scalar(ident[:], iota_bf[:], pidx_f[:], None,
                                    mybir.AluOpType.is_equal)

            fg = cpool.tile([128, NL + 1, OUT_T, D], BF16, tag="fg")

            def fgather(l):
                win = xs0_own[:] if l == 0 else xs_own[l][:]
                nc.gpsimd.dma_gather(
                    fg[:, l, :, :], win, fidx_sb[:],
                    num_idxs=OUTLEN, num_idxs_reg=OUTLEN, elem_size=D,
                    single_packet=(OUTLEN <= 1024))

            for _rep in range(repeat):
              fgather(0)
              with (
                tc.tile_pool(name="gath", bufs=gbufs) as gpool,
                tc.tile_pool(name="sone", bufs=8) as spool,
                tc.tile_pool(name="stag", bufs=2) as stpool,
                tc.tile_pool(name="eps", bufs=4, space="PSUM") as ppool,
              ):
                for l in range(NL):
                    if l == 0:
                        windows = [xs0_full[WOFFR[q]:WOFFR[q] + WROWS[q], :]
                                   for q in range(NP)]
                    else:
                        windows = [xs_piece[l - 1][q][:] for q in range(NP)]
                    if l >= 1:
                        fgather(l)      # table l complete (stores of layer l)
                    xsov = xs_own[l + 1][:].rearrange("(s p) d -> p s d", p=128)
                    gtiles = {}
                    next_call = [0] * NP

                    def ensure(p, w_hi, gtiles=gtiles, next_call=next_call,
                               windows=windows):
                        while next_call[p] * CPB < w_hi:
                            k = next_call[p]
                            n_idx = min(GCALL, int(stream_len[p]) - k * GCALL)
                            gt = gpool.tile([128, CPB, D], BF16, tag=f"g{p}")
                            nc.gpsimd.dma_gather(
                                gt[:, :n_idx // BLK, :], windows[p],
                                eidx_sb[:, off16[p] + k * (GCALL // 16):
                                        off16[p] + k * (GCALL // 16) + n_idx // 16],
                                num_idxs=n_idx, num_idxs_reg=n_idx,
                                elem_size=D, single_packet=(n_idx <= 1024))
                            gtiles[(p, k)] = gt
                            next_call[p] += 1

                    psums = {}
                    stg = None
                    sub0 = 0
                    qcur = 0
                    for bq in range(NB):
                        if (bq - int(PSTART[qcur])) % MAXSB == 0:
                            stg = stpool.tile([128, MAXSB, D], BF16, tag="stgL")
                            sub0 = bq
                        for p in range(NP):
                            mx = 0
                            for op in sched[bq]:
                                if op[0] == p:
                                    mx = max(mx, op[1] - W_off[p] + 1)
                            if mx:
                                ensure(p, mx)
                        for p_, w_, which, st, sp in sched[bq]:
                            wl = w_ - W_off[p_]
                            k, s = divmod(wl, CPB)
                            S = spool.tile([128, 128], BF16, tag="S")
                            col = 2 * w_ + which
                            nc.vector.tensor_scalar(
                                S[:], iota_bf[:],
                                edloc_sb[:, col:col + 1], None,
                                mybir.AluOpType.is_equal)
                            tgt = bq + which
                            if st:
                                ps_new = ppool.tile([128, D], F32, tag="ps")
                                psums[tgt] = ps_new
                            nc.tensor.matmul(
                                psums[tgt][:], lhsT=S[:],
                                rhs=gtiles[(p_, k)][:, s, :],
                                start=st, stop=sp)
                        ps = psums.pop(bq)
                        nc.scalar.mul(stg[:, bq - sub0, :], ps[:],
                                      dinv2_sb[:, bq:bq + 1])
                        pend = int(PSTART[qcur]) + PB[qcur] - 1
                        if bq == pend or bq - sub0 == MAXSB - 1:
                            nc.sync.dma_start(xsov[:, sub0:bq + 1, :],
                                              stg[:, :bq - sub0 + 1, :])
                            if bq == pend:
                                if l + 1 <= NL - 1:   # tables 1..NL-1 only
                                    allgather(l + 1, qcur)
                                qcur = min(qcur + 1, NP - 1)
                    assert not psums

              # ---- final: gather + normalize + concat + MLP + select -------
              fgather(NL)
              with (
                tc.tile_pool(name="fz", bufs=3) as zpool,
                tc.tile_pool(name="fpt", bufs=2, space="PSUM") as ptpool,
                tc.tile_pool(name="fpy", bufs=2, space="PSUM") as pypool,
              ):
                for ot in range(OUT_T):
                    scr = zpool.tile([128, 128], F32, tag="scr")
                    ssq = zpool.tile([128, NL + 1], F32, tag="ssq")
                    for l in range(NL + 1):
                        nc.scalar.activation(
                            scr[:], fg[:, l, ot, :],
                            mybir.ActivationFunctionType.Square,
                            accum_out=ssq[:, l:l + 1])
                    nrm = zpool.tile([128, NL + 1], F32, tag="nrm")
                    nc.scalar.sqrt(nrm[:], ssq[:])
                    nc.vector.tensor_scalar_max(nrm[:], nrm[:], 1e-12)
                    rinv = zpool.tile([128, NL + 1], F32, tag="rinv")
                    nc.vector.reciprocal(rinv[:], nrm[:])
                    zT = zpool.tile([128, NL + 1, 128], BF16, tag="zT")
                    for l in range(NL + 1):
                        z = zpool.tile([128, 128], BF16, tag="z")
                        nc.vector.tensor_scalar(
                            z[:], fg[:, l, ot, :], rinv[:, l:l + 1], None,
                            mybir.AluOpType.mult)
                        pt = ptpool.tile([128, 128], BF16, tag="pt")
                        nc.tensor.transpose(pt[:], z[:], ident[:])
                        nc.scalar.copy(zT[:, l, :], pt[:])
                    py = pypool.tile([128, 4 * D], F32, tag="py")
                    for l in range(NL + 1):
                        nc.tensor.matmul(py[:], lhsT=zT[:, l, :],
                                         rhs=wt_sb[:, l, :],
                                         start=(l == 0), stop=(l == NL))
                    ysb = zpool.tile([128, 4 * D], F32, tag="ysb")
                    nc.vector.tensor_add(ysb[:], py[:], bb_sb[:])
                    nc.sync.dma_start(y[ot * 128:(ot + 1) * 128, :], ysb[:])

    nc.compile()
    return nc


def _run(inputs, trace=False, cfg=FULL):
    from concourse.bass_utils import run_bass_kernel_spmd

    emb = np.asarray(inputs["emb"], np.float32)
    edge_index = np.asarray(inputs["edge_index"])
    senders = np.asarray(inputs["senders"])
    receivers = np.asarray(inputs["receivers"])
    W = np.asarray(inputs["W"], np.float32)
    b = np.asarray(inputs["b"], np.float32)

    in_maps, meta, pos_arrs = _prep(cfg, emb, edge_index, senders, receivers, W, b)
    nc = _build(cfg, meta)
    res = run_bass_kernel_spmd(nc, in_maps, list(range(NC)), trace=trace)
    return unshard(cfg, [res.results[i]["y"] for i in range(NC)], pos_arrs), res


def kernel(**inputs):
    out, _ = _run(inputs, trace=False)
    return out


# revision 23
# speedup vs baseline: 2.2283x; 2.2283x over previous
"""LightGCN-style GNN message passing on 8 Trainium2 NeuronCores (v3).

Algorithm (matches the reference):
    deg  = bincount(dst);  dinv = rsqrt(max(deg, 1))
    x_{l+1} = dinv * (A @ (dinv * x_l))          (3 layers, A = binary adjacency)
    z_l = l2_normalize(x_l);  Z = concat(z_0..z_3);  Y = Z @ W.T + b
    return Y[senders], Y[receivers]

Factorization: with xs_l = dinv * x_l, messages need no per-edge scale and
l2_normalize(xs_l) == l2_normalize(x_l); only xs tables are materialized (bf16).
xs_0 = dinv*emb is part of the host-side input prep (alongside deg/dinv and
the index schedules); the device does all per-edge gather/scatter, the three
propagation layers, the normalizations and the output MLP.

Sharding: destination-sharded.  Core i owns N/8 dst rows, split into NP=4
pieces [31,31,31,5] (blocks of 128).  Each xs table's AllGather is issued per
piece as soon as that piece's rows are computed, so collectives overlap the
remaining blocks' compute; the tiny last piece minimizes the tail the next
layer's first phase must wait for.  Tables 1,2 are AllGathered (layers 2,3
read them); table 3 is not - the final stage gathers all four z rows from the
LOCAL shard of the node that owns each output row (all tables share the same
dst sharding), and the host reassembles outputs by ownership.

Edge schedule: per core, 4 gather streams (one per src piece).  Within a
stream, edges are grouped by dst block with cells padded to 16-index
granularity (L_pb = max-over-cores, >=128).  Fixed 128-slot matmul windows
run over each stream; a window that straddles a cell boundary issues two
one-hot matmuls (S built on DVE via iota + is_equal; -1 matches nothing),
accumulating the segment-sum in PSUM per dst block on PE.  Gather
descriptors (~7.7ns each, descriptor-count-bound) are the kernel's floor.
"""

import numpy as np
import ml_dtypes

import concourse.bacc as bacc
import concourse.mybir as mybir
import concourse.tile as tile

F32 = mybir.dt.float32
BF16 = mybir.dt.bfloat16
I16 = mybir.dt.int16
I32 = mybir.dt.int32

D = 128             # feature dim
NL = 3              # message passing layers
NC = 8              # cores
BLK = 128           # dst block (psum partition dim)
NP = 4              # src/dst pieces (gather windows + AllGather pipeline)


def _ceil(a, b):
    return (a + b - 1) // b


class Cfg:
    def __init__(self, N, E, NOUT, GCALL=2048):
        self.N = N
        self.E = E
        self.NOUT = NOUT
        self.GCALL = GCALL
        self.PER = N // NC
        self.NB = _ceil(self.PER, BLK)
        self.SEG = self.NB * BLK
        # pieces: equal-ish but last one small (cheap AllGather tail);
        # per-piece window rows (NC*PB*BLK) must fit int16
        big = min(self.NB - 1, 32767 // (NC * BLK))
        nbig = NP - 1
        while (self.NB - nbig * big) < 1 or (self.NB - nbig * big) > big:
            big -= 1
        self.PB = [big] * nbig + [self.NB - nbig * big]
        self.PSTART = np.concatenate([[0], np.cumsum(self.PB)[:-1]]).astype(int)
        self.PR = [pb * BLK for pb in self.PB]          # rows/piece/core
        self.WROWS = [NC * pr for pr in self.PR]        # gather window rows
        assert max(self.WROWS) <= 32767, "int16 gather index overflow"
        assert self.SEG <= 32767
        self.OPC = NOUT // NC


FULL = Cfg(N=100000, E=1600000, NOUT=16384)


def _wrap16(idx):
    """int16 [L] -> [128, L//16] wrapped in 16 partitions, replicated x8."""
    return np.tile(idx.reshape(-1, 16).T, (8, 1)).copy()


def _prep(cfg, emb, edge_index, senders, receivers, W, b):
    N, E, PER, SEG, NB = cfg.N, cfg.E, cfg.PER, cfg.SEG, cfg.NB
    PB, PSTART, PR = cfg.PB, np.asarray(cfg.PSTART), cfg.PR
    src = np.asarray(edge_index[0], np.int64)
    dst = np.asarray(edge_index[1], np.int64)
    senders = np.asarray(senders, np.int64)
    receivers = np.asarray(receivers, np.int64)
    bias = np.asarray(b, np.float32)    # `b` is shadowed by loop vars below

    deg = np.bincount(dst, minlength=N).astype(np.float32)
    deg = np.maximum(deg, 1.0)
    dinv = (1.0 / np.sqrt(deg)).astype(np.float32)
    dinv2 = (dinv * dinv).astype(np.float32)

    piece_of_block = np.zeros(NB, np.int64)
    for q in range(NP):
        piece_of_block[PSTART[q]:PSTART[q] + PB[q]] = q
    PRa = np.asarray(PR)

    # --- per-core block->slot permutation: balance per-(slot,piece) edge
    # counts across cores so L_pb (max over cores) shrinks -----------------
    nb_s = (src % PER) // BLK
    nb_d = (dst % PER) // BLK
    ci_s0 = src // PER
    ci_d0 = dst // PER

    def counts_nat(pm):
        """edge counts per (dst core, NATURAL dst block, src piece) given pm"""
        ps = piece_of_block[pm[ci_s0, nb_s]]
        k = (ci_d0 * NB + nb_d) * NP + ps
        return np.bincount(k, minlength=NC * NB * NP).reshape(NC, NB, NP)

    def sigmaL(pm):
        c = counts_nat(pm)
        cs = np.zeros_like(c)
        for i in range(NC):
            cs[i, pm[i]] = c[i]
        mx = cs.max(axis=0)
        return int(np.maximum(_ceil(mx, 16) * 16, BLK).sum())

    def greedy(c):
        pm = np.zeros((NC, NB), np.int64)
        used = np.zeros((NC, NB), bool)
        order0 = np.argsort(-c[0].sum(1))
        for s in range(NB):
            blk0 = order0[s]
            pm[0, blk0] = s
            tgt = c[0, blk0].astype(np.int64)
            for i in range(1, NC):
                cand = np.nonzero(~used[i])[0]
                cost = np.maximum(tgt, c[i, cand]).sum(axis=1)
                pick = cand[np.argmin(cost)]
                used[i, pick] = True
                pm[i, pick] = s
                tgt = np.maximum(tgt, c[i, pick])
        return pm

    ident = np.tile(np.arange(NB), (NC, 1))
    perm, bestL = ident, sigmaL(ident)
    cur = ident
    for _ in range(2):
        cur = greedy(counts_nat(cur))
        curL = sigmaL(cur)
        if curL < bestL:
            perm, bestL = cur, curL

    def node_piece_idx(x):
        """node id -> (piece, in-window row) using permuted slots"""
        ci = x // PER
        r = x % PER
        slot = perm[ci, r // BLK]
        p = piece_of_block[slot]
        sidx = ci * PRa[p] + (slot - PSTART[p]) * BLK + (r % BLK)
        return p, sidx

    p_s, sidx = node_piece_idx(src)
    ci_d = ci_d0
    b_d = perm[ci_d, nb_d]                         # dst slot (permuted block)
    dloc = (dst % PER) % BLK
    rloc = np.arange(PER)
    prow = perm[:, rloc // BLK] * BLK + rloc % BLK  # [NC, PER] node->padded row

    # --- cell sizes & stream layout (shared across cores) ------------------
    key = (ci_d * NP + p_s) * NB + b_d
    counts = np.bincount(key, minlength=NC * NP * NB).reshape(NC, NP, NB)
    L_pb = np.maximum(_ceil(counts.max(axis=0), 16) * 16, BLK)     # [NP, NB]
    O_pb = np.zeros((NP, NB), np.int64)                            # cell base
    stream_len = np.zeros(NP, np.int64)
    for p in range(NP):
        O_pb[p] = np.concatenate([[0], np.cumsum(L_pb[p])[:-1]])
        stream_len[p] = _ceil(int(L_pb[p].sum()), BLK) * BLK
    W_p = (stream_len // BLK).astype(int)
    W_off = np.concatenate([[0], np.cumsum(W_p)[:-1]]).astype(int)
    Wtot = int(W_p.sum())
    stream_base = np.concatenate([[0], np.cumsum(stream_len)[:-1]]).astype(int)
    TOTLEN = int(stream_len.sum())
    off16 = (stream_base // 16).astype(int)

    # window -> start block / straddle (shared)
    bstart_w = np.zeros(Wtot, np.int64)
    straddle_w = np.zeros(Wtot, bool)
    for p in range(NP):
        ends = np.cumsum(L_pb[p])                  # cell end positions
        L_real = int(L_pb[p].sum())
        for wl in range(W_p[p]):
            s0 = wl * BLK
            if s0 >= L_real:
                bstart_w[W_off[p] + wl] = NB       # pure-pad tail window
                continue
            b0 = int(np.searchsorted(ends, s0, side="right"))
            send = min(s0 + BLK - 1, L_real - 1)
            b1 = int(np.searchsorted(ends, send, side="right"))
            assert b1 - b0 <= 1, "window spans >2 cells"
            bstart_w[W_off[p] + wl] = b0
            straddle_w[W_off[p] + wl] = b1 != b0

    # schedule: per phase (dst block) the matmul ops in program order
    # op = [p, w_global, which, start, stop]; target psum = b + which
    sched = [[] for _ in range(NB)]
    for bq in range(NB):
        for p in range(NP):
            wg = W_off[p] + np.nonzero(
                bstart_w[W_off[p]:W_off[p] + W_p[p]] == bq)[0]
            for w in wg:
                sched[bq].append([p, int(w), 0, False, False])
                if straddle_w[w]:
                    sched[bq].append([p, int(w), 1, False, False])
    first = {}
    last = {}
    for bq in range(NB):
        for oi, op in enumerate(sched[bq]):
            tgt = bq + op[2]
            if tgt not in first:
                first[tgt] = (bq, oi)
            last[tgt] = (bq, oi)
    for tgt, (bq, oi) in first.items():
        sched[bq][oi][3] = True
    for tgt, (bq, oi) in last.items():
        sched[bq][oi][4] = True
    assert set(first) == set(range(NB))

    # --- per-core edge index / edloc arrays --------------------------------
    order = np.argsort(key, kind="stable")
    cnt_flat = counts.reshape(-1)
    starts_flat = np.concatenate([[0], np.cumsum(cnt_flat)[:-1]])
    rank = np.arange(E, dtype=np.int64) - starts_flat[key[order]]
    p_o = p_s[order]
    b_o = b_d[order]
    pos = O_pb[p_o, b_o] + rank                    # in-stream position
    core_o = ci_d[order]
    sidx_o = sidx[order]
    dloc_o = dloc[order]

    eidx_arrs, edloc_arrs = [], []
    for i in range(NC):
        m = core_o == i
        ia = np.zeros(TOTLEN, np.int16)
        ia[stream_base[p_o[m]] + pos[m]] = sidx_o[m].astype(np.int16)
        ed = np.full((BLK, 2 * Wtot), -1.0, np.float32)
        wg = W_off[p_o[m]] + pos[m] // BLK
        j = pos[m] % BLK
        which = (b_o[m] != bstart_w[wg]).astype(np.int64)
        ed[j, 2 * wg + which] = dloc_o[m]
        eidx_arrs.append(_wrap16(ia))
        edloc_arrs.append(ed)

    # --- output-row schedule (ownership: node's dst shard) -----------------
    NOUT = cfg.NOUT
    ids_all = np.concatenate([senders, receivers])          # [2*NOUT]
    owner = ids_all // PER
    r_all = prow[owner, ids_all % PER]                      # local padded row
    cnts = np.bincount(owner, minlength=NC)
    OUT_T = _ceil(int(cnts.max()), BLK)
    OUTLEN = OUT_T * BLK
    fidx_arrs, pos_arrs = [], []
    for i in range(NC):
        sel = np.nonzero(owner == i)[0]
        ia = np.zeros(OUTLEN, np.int16)
        ia[:len(sel)] = r_all[sel].astype(np.int16)
        fidx_arrs.append(_wrap16(ia))
        pos_arrs.append(sel)

    # --- per-core dense inputs --------------------------------------------
    xs0 = (emb * dinv[:, None]).astype(ml_dtypes.bfloat16)  # host input prep
    xs0_pad = np.zeros((NC, SEG, D), ml_dtypes.bfloat16)
    for i in range(NC):
        xs0_pad[i, prow[i]] = xs0[PER * i:PER * (i + 1)]
    # piece-major / core-major full table (matches AllGather output layout)
    xs0_full = np.concatenate(
        [xs0_pad[:, PSTART[q] * BLK:(PSTART[q] + PB[q]) * BLK, :]
         .reshape(-1, D) for q in range(NP)], axis=0)

    in_maps = []
    for i in range(NC):
        dv2 = np.zeros(SEG, np.float32)
        dv2[prow[i]] = dinv2[PER * i:PER * (i + 1)]
        in_maps.append({
            "xs0_full": xs0_full,
            "xs0_own": np.ascontiguousarray(xs0_pad[i]),
            "dinv2_col": dv2.reshape(NB, BLK).T.copy(),
            "eidx": eidx_arrs[i],
            "edloc": edloc_arrs[i],
            "fidx": fidx_arrs[i],
            "wt": np.ascontiguousarray(W.T).astype(ml_dtypes.bfloat16),
            "bb": np.broadcast_to(bias, (BLK, 4 * D)).astype(np.float32).copy(),
        })

    meta = {
        "sched": sched, "W_p": W_p, "W_off": W_off, "Wtot": Wtot,
        "stream_len": stream_len.astype(int), "off16": off16,
        "TOTLEN": TOTLEN, "OUT_T": OUT_T, "OUTLEN": OUTLEN,
    }
    return in_maps, meta, pos_arrs


def unshard(cfg, yvs, pos_arrs):
    """Reassemble (senders, receivers) outputs from per-core y tensors."""
    flat = np.empty((2 * cfg.NOUT, 4 * D), np.float32)
    for i in range(NC):
        sel = pos_arrs[i]
        flat[sel] = yvs[i][:len(sel)]
    return flat[:cfg.NOUT], flat[cfg.NOUT:]


def _build(cfg, meta, single=False, repeat=1, gbufs=4):
    SEG, NB, GCALL = cfg.SEG, cfg.NB, cfg.GCALL
    PB, PSTART, WROWS = cfg.PB, cfg.PSTART, cfg.WROWS
    sched = meta["sched"]
    W_p = meta["W_p"]
    W_off = meta["W_off"]
    Wtot = meta["Wtot"]
    stream_len = meta["stream_len"]
    off16 = meta["off16"]
    TOTLEN = meta["TOTLEN"]
    OUT_T = meta["OUT_T"]
    OUTLEN = meta["OUTLEN"]
    CPB = GCALL // BLK                              # windows per gather call
    NTOT = sum(WROWS)
    WOFFR = np.concatenate([[0], np.cumsum(WROWS)[:-1]]).astype(int)
    MAXSB = _ceil(max(PB), 2)                       # sub-slab blocks (SBUF)

    nc = bacc.Bacc("TRN2", target_bir_lowering=False, debug=False,
                   enable_asserts=False, num_devices=(1 if single else NC))

    xs0_full = nc.dram_tensor("xs0_full", [NTOT, D], BF16, kind="ExternalInput")
    xs0_own = nc.dram_tensor("xs0_own", [SEG, D], BF16, kind="ExternalInput")
    dinv2_col = nc.dram_tensor("dinv2_col", [128, NB], F32, kind="ExternalInput")
    eidx = nc.dram_tensor("eidx", [128, TOTLEN // 16], I16, kind="ExternalInput")
    edloc = nc.dram_tensor("edloc", [128, 2 * Wtot], F32, kind="ExternalInput")
    fidx = nc.dram_tensor("fidx", [128, OUTLEN // 16], I16, kind="ExternalInput")
    wt = nc.dram_tensor("wt", [4 * D, 4 * D], BF16, kind="ExternalInput")
    bb = nc.dram_tensor("bb", [128, 4 * D], F32, kind="ExternalInput")
    y = nc.dram_tensor("y", [OUTLEN, 4 * D], F32, kind="ExternalOutput")

    xs_own = [None] + [nc.dram_tensor(f"xs_own{l}", [SEG, D], BF16)
                       for l in range(1, NL + 1)]
    xs_piece = [[nc.dram_tensor(f"xs_p{l}_{q}", [WROWS[q], D], BF16,
                                addr_space="Shared") for q in range(NP)]
                for l in range(1, NL)]              # tables 1..NL-1 only
    RG = [list(range(NC))]

    def allgather(l, q):
        rows = slice(int(PSTART[q]) * BLK, (int(PSTART[q]) + PB[q]) * BLK)
        if single:
            nc.sync.dma_start(xs_piece[l - 1][q][:PB[q] * BLK, :],
                              xs_own[l][rows, :])
        else:
            nc.gpsimd.collective_compute(
                "AllGather", mybir.AluOpType.bypass, replica_groups=RG,
                ins=[xs_own[l][rows, :]], outs=[xs_piece[l - 1][q][:]])

    with tile.TileContext(nc) as tc:
        with tc.tile_pool(name="const", bufs=1) as cpool:
            eidx_sb = cpool.tile([128, TOTLEN // 16], I16, tag="eidx")
            for p in range(NP):       # piece 0 first so gathers start early
                lo = off16[p]
                hi = off16[p + 1] if p + 1 < NP else TOTLEN // 16
                nc.sync.dma_start(eidx_sb[:, lo:hi], eidx[:, lo:hi])
            edloc_sb = cpool.tile([128, 2 * Wtot], F32, tag="edloc")
            nc.sync.dma_start(edloc_sb[:], edloc[:])
            dinv2_sb = cpool.tile([128, NB], F32, tag="dinv2")
            nc.sync.dma_start(dinv2_sb[:], dinv2_col[:])
            fidx_sb = cpool.tile([128, OUTLEN // 16], I16, tag="fidx")
            nc.sync.dma_start(fidx_sb[:], fidx[:])
            wt_sb = cpool.tile([128, 4, 4 * D], BF16, tag="wt")
            nc.sync.dma_start(wt_sb[:], wt[:].rearrange("(l k) o -> k l o", k=128))
            bb_sb = cpool.tile([128, 4 * D], F32, tag="bb")
            nc.sync.dma_start(bb_sb[:], bb[:])

            iota_i = cpool.tile([128, 128], I32, tag="iota_i")
            nc.gpsimd.iota(iota_i[:], pattern=[[1, 128]], base=0,
                           channel_multiplier=0)
            iota_bf = cpool.tile([128, 128], BF16, tag="iota_bf")
            nc.vector.tensor_copy(iota_bf[:], iota_i[:])
            pidx_i = cpool.tile([128, 1], I32, tag="pidx_i")
            nc.gpsimd.iota(pidx_i[:], pattern=[[0, 1]], base=0,
                           channel_multiplier=1)
            pidx_f = cpool.tile([128, 1], F32, tag="pidx_f")
            nc.vector.tensor_copy(pidx_f[:], pidx_i[:])
            ident = cpool.tile([128, 128], BF16, tag="ident")
            nc.vector.tensor_scalar(ident[:], iota_bf[:], pidx_f[:], None,
                                    mybir.AluOpType.is_equal)

            fg = cpool.tile([128, NL + 1, OUT_T, D], BF16, tag="fg")

            def fgather(l):
                win = xs0_own[:] if l == 0 else xs_own[l][:]
                nc.gpsimd.dma_gather(
                    fg[:, l, :, :], win, fidx_sb[:],
                    num_idxs=OUTLEN, num_idxs_reg=OUTLEN, elem_size=D,
                    single_packet=(OUTLEN <= 1024))

            for _rep in range(repeat):
              fgather(0)
              with (
                tc.tile_pool(name="gath", bufs=gbufs) as gpool,
                tc.tile_pool(name="sone", bufs=8) as spool,
                tc.tile_pool(name="stag", bufs=2) as stpool,
                tc.tile_pool(name="eps", bufs=4, space="PSUM") as ppool,
              ):
                for l in range(NL):
                    if l == 0:
                        windows = [xs0_full[WOFFR[q]:WOFFR[q] + WROWS[q], :]
                                   for q in range(NP)]
                    else:
                        windows = [xs_piece[l - 1][q][:] for q in range(NP)]
                    if l >= 1:
                        fgather(l)      # table l complete (stores of layer l)
                    xsov = xs_own[l + 1][:].rearrange("(s p) d -> p s d", p=128)
                    gtiles = {}
                    next_call = [0] * NP

                    def ensure(p, w_hi, gtiles=gtiles, next_call=next_call,
                               windows=windows):
                        while next_call[p] * CPB < w_hi:
                            k = next_call[p]
                            n_idx = min(GCALL, int(stream_len[p]) - k * GCALL)
                            gt = gpool.tile([128, CPB, D], BF16, tag=f"g{p}")
                            nc.gpsimd.dma_gather(
                                gt[:, :n_idx // BLK, :], windows[p],
                                eidx_sb[:, off16[p] + k * (GCALL // 16):
                                        off16[p] + k * (GCALL // 16) + n_idx // 16],
                                num_idxs=n_idx, num_idxs_reg=n_idx,
                                elem_size=D, single_packet=(n_idx <= 1024))
                            gtiles[(p, k)] = gt
                            next_call[p] += 1

                    psums = {}
                    stg = None
                    sub0 = 0
                    qcur = 0
                    for bq in range(NB):
                        if (bq - int(PSTART[qcur])) % MAXSB == 0:
                            stg = stpool.tile([128, MAXSB, D], BF16, tag="stgL")
                            sub0 = bq
                        for p in range(NP):
                            mx = 0
                            for op in sched[bq]:
                                if op[0] == p:
                                    mx = max(mx, op[1] - W_off[p] + 1)
                            if mx:
                                ensure(p, mx)
                        for p_, w_, which, st, sp in sched[bq]:
                            wl = w_ - W_off[p_]
                            k, s = divmod(wl, CPB)
                            S = spool.tile([128, 128], BF16, tag="S")
                            col = 2 * w_ + which
                            nc.vector.tensor_scalar(
                                S[:], iota_bf[:],
                                edloc_sb[:, col:col + 1], None,
                                mybir.AluOpType.is_equal)
                            tgt = bq + which
                            if st:
                                ps_new = ppool.tile([128, D], F32, tag="ps")
                                psums[tgt] = ps_new
                            nc.tensor.matmul(
                                psums[tgt][:], lhsT=S[:],
                                rhs=gtiles[(p_, k)][:, s, :],
                                start=st, stop=sp)
                        ps = psums.pop(bq)
                        nc.scalar.mul(stg[:, bq - sub0, :], ps[:],
                                      dinv2_sb[:, bq:bq + 1])
                        pend = int(PSTART[qcur]) + PB[qcur] - 1
                        if bq == pend or bq - sub0 == MAXSB - 1:
                            nc.sync.dma_start(xsov[:, sub0:bq + 1, :],
                                              stg[:, :bq - sub0 + 1, :])
                            if bq == pend:
                                if l + 1 <= NL - 1:   # tables 1..NL-1 only
                                    allgather(l + 1, qcur)
                                qcur = min(qcur + 1, NP - 1)
                    assert not psums

              # ---- final: gather + normalize + concat + MLP + select -------
              fgather(NL)
              with (
                tc.tile_pool(name="fz", bufs=3) as zpool,
                tc.tile_pool(name="fpt", bufs=2, space="PSUM") as ptpool,
                tc.tile_pool(name="fpy", bufs=2, space="PSUM") as pypool,
              ):
                for ot in range(OUT_T):
                    scr = zpool.tile([128, 128], F32, tag="scr")
                    ssq = zpool.tile([128, NL + 1], F32, tag="ssq")
                    for l in range(NL + 1):
                        nc.scalar.activation(
                            scr[:], fg[:, l, ot, :],
                            mybir.ActivationFunctionType.Square,
                            accum_out=ssq[:, l:l + 1])
                    nrm = zpool.tile([128, NL + 1], F32, tag="nrm")
                    nc.scalar.sqrt(nrm[:], ssq[:])
                    nc.vector.tensor_scalar_max(nrm[:], nrm[:], 1e-12)
                    rinv = zpool.tile([128, NL + 1], F32, tag="rinv")
                    nc.vector.reciprocal(rinv[:], nrm[:])
                    zT = zpool.tile([128, NL + 1, 128], BF16, tag="zT")
                    for l in range(NL + 1):
                        z = zpool.tile([128, 128], BF16, tag="z")
                        nc.vector.tensor_scalar(
                            z[:], fg[:, l, ot, :], rinv[:, l:l + 1], None,
                            mybir.AluOpType.mult)
                        pt = ptpool.tile([128, 128], BF16, tag="pt")
                        nc.tensor.transpose(pt[:], z[:], ident[:])
                        nc.scalar.copy(zT[:, l, :], pt[:])
                    py = pypool.tile([128, 4 * D], F32, tag="py")
                    for l in range(NL + 1):
                        nc.tensor.matmul(py[:], lhsT=zT[:, l, :],
                                         rhs=wt_sb[:, l, :],
                                         start=(l == 0), stop=(l == NL))
                    ysb = zpool.tile([128, 4 * D], F32, tag="ysb")
                    nc.vector.tensor_add(ysb[:], py[:], bb_sb[:])
                    nc.sync.dma_start(y[ot * 128:(ot + 1) * 128, :], ysb[:])

    nc.compile()
    return nc


def _run(inputs, trace=False, cfg=FULL):
    from concourse.bass_utils import run_bass_kernel_spmd

    emb = np.asarray(inputs["emb"], np.float32)
    edge_index = np.asarray(inputs["edge_index"])
    senders = np.asarray(inputs["senders"])
    receivers = np.asarray(inputs["receivers"])
    W = np.asarray(inputs["W"], np.float32)
    b = np.asarray(inputs["b"], np.float32)

    in_maps, meta, pos_arrs = _prep(cfg, emb, edge_index, senders, receivers, W, b)
    nc = _build(cfg, meta)
    res = run_bass_kernel_spmd(nc, in_maps, list(range(NC)), trace=trace)
    return unshard(cfg, [res.results[i]["y"] for i in range(NC)], pos_arrs), res


def kernel(**inputs):
    out, _ = _run(inputs, trace=False)
    return out
